# revision 21
# baseline (speedup 1.0000x reference)
"""Causal self-attention on 8 trn2 NeuronCores.

Sharding: core c -> (batch b = c//2, head-group g = c%2).  Each head-group
is 8 heads = 512 channels.  Per core:
  - q/k/v projections of x[b] restricted to the group's 512 columns
  - causal attention for the 8 heads, computed in the transposed
    orientation S^T = [tk, tq] so softmax denominators come from a
    ones-column appended to V (AV matmul yields them for free) and no
    transposes are needed anywhere
  - partial output projection through the group's 512 rows of Wo
Host sums the two partials per batch and adds (bv @ Wo + bo): softmax
weights sum to one, so the v-bias passes through attention additively.

The QK stationary operand is zero-padded to a full [128,128] footprint
(kz holds each head's k tile in its 64 q-channel rows, zeros in the other
head's rows, so the full 128-partition q pair streams through) — without
this the PE activity monitor keeps the tensor engine clock-gated at
1.2 GHz for the whole attention phase.
"""

import numpy as np
import ml_dtypes

import concourse.bass as bass
import concourse.mybir as mybir
from concourse import bacc, tile
from concourse.bass_utils import run_bass_kernel_spmd

B, T, C, H = 4, 2048, 1024, 16
HD = C // H          # 64
G = 2                # head groups (cores per batch)
HG = H // G          # 8 heads per group
CG = C // G          # 512 channels per group
CGP = CG // 128      # 4 c_out tiles per group
P = 128
W = 512              # free-dim window (one PSUM bank of f32)
NW = T // W          # 4 windows
NTT = T // P         # 16 t tiles
NCI = C // P         # 8 c_in chunks
VS = HD + 1          # 65: v plus ones column

_cached_nc = None


def _build():
    f32 = mybir.dt.float32
    f32r = mybir.dt.float32r
    bf16 = mybir.dt.bfloat16
    AF = mybir.ActivationFunctionType
    nc = bacc.Bacc("TRN2", target_bir_lowering=False, debug=False, num_devices=8)

    xt_d = nc.dram_tensor("xt", [C, T], bf16, kind="ExternalInput")
    wq_d = nc.dram_tensor("wq", [C, CG], bf16, kind="ExternalInput")
    wk_d = nc.dram_tensor("wk", [C, CG], bf16, kind="ExternalInput")
    wv_d = nc.dram_tensor("wv", [C, CG], bf16, kind="ExternalInput")
    wo_d = nc.dram_tensor("wo", [CG, C], f32r, kind="ExternalInput")
    bq_d = nc.dram_tensor("bq", [P, CGP], f32, kind="ExternalInput")
    bk_d = nc.dram_tensor("bk", [P, CGP], f32, kind="ExternalInput")
    mask_d = nc.dram_tensor("mask", [P, P], f32, kind="ExternalInput")
    mask2_d = nc.dram_tensor("mask2", [P, 2 * P], f32, kind="ExternalInput")
    out_d = nc.dram_tensor("outp", [C, T], f32, kind="ExternalOutput")

    mm = lambda out, lhsT, rhs, start, stop: nc.tensor.matmul(
        out, lhsT, rhs, start=start, stop=stop)

    with tile.TileContext(nc) as tc:
        with (
            tc.tile_pool(name="pers", bufs=1) as pers,
            tc.tile_pool(name="attn", bufs=1) as attn,
            tc.tile_pool(name="pt", bufs=6) as ptpool,
            tc.tile_pool(name="dn", bufs=2) as dnpool,
            tc.tile_pool(name="rb", bufs=2) as rbpool,
            tc.tile_pool(name="osb", bufs=3) as opool,
            tc.tile_pool(name="psum_b", bufs=2, space="PSUM") as psum_b,
            tc.tile_pool(name="psum_mm", bufs=2, space="PSUM") as psum_mm,
            tc.tile_pool(name="psum_av", bufs=2, space="PSUM") as psum_av,
        ):
            qT = pers.tile([P, CGP, T], bf16)        # q^T: [c_out, t]
            # kz[:, j, a, tk]: head h=2j+a k-tile in its own 64 rows, 0 else
            kz = pers.tile([P, CGP, 2, T], bf16)
            vp = pers.tile([P, NTT, HG * VS], bf16)  # v rows + ones col/head
            wo_sb = pers.tile([P, CGP, C], f32r)
            mask_sb = pers.tile([P, P], f32)
            mask2_sb = pers.tile([P, 2 * P], f32)
            bq_sb = pers.tile([P, CGP], f32)
            bk_sb = pers.tile([P, CGP], f32)
            ones_sb = pers.tile([P, HG], f32)

            nc.sync.dma_start(out=wo_sb, in_=wo_d.ap().rearrange("(c p) n -> p c n", p=P))
            nc.sync.dma_start(out=mask_sb, in_=mask_d.ap())
            nc.sync.dma_start(out=mask2_sb, in_=mask2_d.ap())
            nc.sync.dma_start(out=bq_sb, in_=bq_d.ap())
            nc.sync.dma_start(out=bk_sb, in_=bk_d.ap())
            nc.vector.memset(ones_sb, 1.0)
            # zero halves of kz that the k-projection copies never touch
            nc.vector.memset(kz[0:HD, :, 1, :], 0.0)
            nc.vector.memset(kz[HD:P, :, 0, :], 0.0)

            # ---- phase B: projections, single pass over all 8 c_in chunks ----
            with (
                tc.tile_pool(name="xchunk", bufs=NCI) as xpool,
                tc.tile_pool(name="wchunk", bufs=1) as wpool,
            ):
                wh = wpool.tile([P, 3, NCI, CG], bf16)
                for wi, wd in enumerate((wq_d, wk_d, wv_d)):
                    nc.sync.dma_start(
                        out=wh[:, wi, :, :],
                        in_=wd.ap().rearrange("(c p) n -> p c n", p=P),
                    )
                xc = []
                for ci in range(NCI):
                    t_ = xpool.tile([P, T], bf16, tag="xc")
                    # two half-tile DMAs so the first matmuls start sooner
                    nc.sync.dma_start(out=t_[:, 0:T // 2],
                                      in_=xt_d.ap()[ci * P:(ci + 1) * P, 0:T // 2])
                    nc.sync.dma_start(out=t_[:, T // 2:],
                                      in_=xt_d.ap()[ci * P:(ci + 1) * P, T // 2:])
                    xc.append(t_)

                def emit_qk(j, w):
                    ws = slice(w * W, (w + 1) * W)
                    psq = psum_b.tile([P, W], f32, tag="psb")
                    for i in range(NCI):
                        mm(psq, wh[:, 0, i, j * P:(j + 1) * P],
                           xc[i][:, ws], start=(i == 0), stop=(i == NCI - 1))
                    nc.scalar.activation(qT[:, j, ws], psq,
                                         AF.Identity, bias=bq_sb[:, j:j + 1])
                    psk = psum_b.tile([P, W], f32, tag="psb")
                    for i in range(NCI):
                        mm(psk, wh[:, 1, i, j * P:(j + 1) * P],
                           xc[i][:, ws], start=(i == 0), stop=(i == NCI - 1))
                    for a in range(2):
                        rows = slice(a * HD, (a + 1) * HD)
                        nc.scalar.activation(kz[rows, j, a, ws], psk[rows, :],
                                             AF.Identity, bias=bk_sb[rows, j:j + 1])

                def emit_v(it):
                    # v in row layout [t, c_out], strided into vp per head
                    psv = psum_b.tile([P, CG], f32, tag="psb")
                    for i in range(NCI):
                        mm(psv, xc[i][:, it * P:(it + 1) * P], wh[:, 2, i, :],
                           start=(i == 0), stop=(i == NCI - 1))
                    v_view = vp[:, it, :].rearrange("p (h x) -> p h x", x=VS)[:, :, 0:HD]
                    nc.scalar.copy(v_view, psv.rearrange("p (h x) -> p h x", x=HD))

                # window-major so early attention windows can start while the
                # projection tail still runs
                for w in range(NW):
                    for it in range(4 * w, 4 * w + 4):
                        emit_v(it)
                    for j in range(CGP):
                        emit_qk(j, w)
            for it in range(NTT):
                ones_view = vp[:, it, :].rearrange("p (h x) -> p h x", x=VS)[:, :, HD:VS]
                nc.vector.tensor_copy(
                    ones_view, ones_sb.rearrange("p (h x) -> p h x", x=1))
            # ---- phase C+D: attention with interleaved output projection ----
            if True:
                yT = attn.tile([P, CGP, T], f32r)
                escale = 1.0 / float(np.sqrt(HD))
                for w in range(NW):
                    tq0 = w * W
                    ntk = (w + 1) * (W // P)
                    for h in range(HG):
                        j, a = h // 2, h % 2
                        ps_av = psum_av.tile([VS, W], f32, tag="av")
                        # tk blocks go in pairs sharing one psum pair-tile and
                        # one exp; a diagonal second block is extended to the
                        # pair's region and cleaned up by mask2 (128 zero cols
                        # + 128 triangular cols)
                        for x in range(ntk // 2):
                            i0, i1 = 2 * x, 2 * x + 1
                            vs0 = max(tq0, i0 * P)
                            n0 = tq0 + W - vs0
                            ps_s = psum_mm.tile([P, 2, W], f32, tag="ps")
                            pt = ptpool.tile([P, 2, W], bf16, tag="pt")
                            mm(ps_s[:, 0, W - n0:], kz[:, j, a, i0 * P:(i0 + 1) * P],
                               qT[:, j, vs0:vs0 + n0], start=True, stop=True)
                            mm(ps_s[:, 1, W - n0:], kz[:, j, a, i1 * P:(i1 + 1) * P],
                               qT[:, j, vs0:vs0 + n0], start=True, stop=True)
                            nc.scalar.activation(pt[:, :, W - n0:], ps_s[:, :, W - n0:],
                                                 AF.Exp, scale=escale)
                            if i0 * P >= tq0:  # diagonal pair
                                nc.vector.tensor_mul(
                                    pt[:, 0, W - n0:W - n0 + P],
                                    pt[:, 0, W - n0:W - n0 + P], mask_sb)
                                nc.vector.tensor_mul(
                                    pt[:, 1, W - n0:W - n0 + 2 * P],
                                    pt[:, 1, W - n0:W - n0 + 2 * P], mask2_sb)
                            mm(ps_av[:, vs0 - tq0:], vp[:, i0, h * VS:(h + 1) * VS],
                               pt[:, 0, W - n0:], start=(i0 == 0), stop=False)
                            mm(ps_av[:, vs0 - tq0:], vp[:, i1, h * VS:(h + 1) * VS],
                               pt[:, 1, W - n0:], start=False, stop=(i1 == ntk - 1))
                        dn = dnpool.tile([1, W], f32, tag="dn")
                        nc.vector.tensor_copy(dn, ps_av[HD:VS, :])
                        rb = rbpool.tile([HD, W], f32, tag="rb")
                        nc.gpsimd.partition_broadcast(rb, dn)
                        nc.vector.reciprocal_approx_fast(out=rb, in_=rb)
                        nc.vector.tensor_mul(
                            yT[a * HD:(a + 1) * HD, j, tq0:tq0 + W],
                            ps_av[0:HD, :], rb)
                    # output projection for this window
                    for m in range(C // P):
                        po = psum_b.tile([P, W], f32, tag="psb")
                        for i in range(CGP):
                            mm(po, wo_sb[:, i, m * P:(m + 1) * P],
                               yT[:, i, tq0:tq0 + W],
                               start=(i == 0), stop=(i == CGP - 1))
                        ot = opool.tile([P, W], f32, tag="ot")
                        nc.vector.tensor_copy(ot, po)
                        nc.sync.dma_start(
                            out=out_d.ap()[m * P:(m + 1) * P, tq0:tq0 + W],
                            in_=ot)

    nc.compile()
    return nc


def get_nc():
    global _cached_nc
    if _cached_nc is None:
        _cached_nc = _build()
    return _cached_nc


def make_in_maps(x, Wq, bq, Wk, bk, Wv, bv, Wo, bo):
    x = np.asarray(x, np.float32)
    mask = np.triu(np.ones((P, P), np.float32))
    mask2 = np.concatenate([np.zeros((P, P), np.float32), mask], axis=1)
    in_maps = []
    for c in range(8):
        b, g = c // 2, c % 2
        cs = slice(g * CG, (g + 1) * CG)
        in_maps.append({
            "xt": np.ascontiguousarray(x[b].T.astype(ml_dtypes.bfloat16)),
            "wq": np.ascontiguousarray(
                np.asarray(Wq, np.float32)[:, cs].astype(ml_dtypes.bfloat16)),
            "wk": np.ascontiguousarray(
                np.asarray(Wk, np.float32)[:, cs].astype(ml_dtypes.bfloat16)),
            "wv": np.ascontiguousarray(
                np.asarray(Wv, np.float32)[:, cs].astype(ml_dtypes.bfloat16)),
            "wo": np.ascontiguousarray(np.asarray(Wo, np.float32)[cs, :]),
            "bq": np.ascontiguousarray(
                np.asarray(bq, np.float32)[cs].reshape(CGP, P).T),
            "bk": np.ascontiguousarray(
                np.asarray(bk, np.float32)[cs].reshape(CGP, P).T),
            "mask": mask,
            "mask2": mask2,
        })
    return in_maps


def combine(results, Wv, bv, Wo, bo):
    const = (np.asarray(bv, np.float32) @ np.asarray(Wo, np.float32)
             + np.asarray(bo, np.float32))
    out = np.empty((B, T, C), np.float32)
    for b in range(B):
        acc = results[2 * b]["outp"] + results[2 * b + 1]["outp"]
        out[b] = acc.T + const[None, :]
    return out


def kernel(x, Wq, bq, Wk, bk, Wv, bv, Wo, bo):
    nc = get_nc()
    in_maps = make_in_maps(x, Wq, bq, Wk, bk, Wv, bv, Wo, bo)
    res = run_bass_kernel_spmd(nc, in_maps, core_ids=list(range(8)))
    return combine(res.results, Wv, bv, Wo, bo)


# revision 22
# speedup vs baseline: 1.1638x; 1.1638x over previous
"""Causal self-attention on 8 trn2 NeuronCores.

Sharding: core c -> (batch b = c//2, head-group g = c%2).  Each head-group
is 8 heads = 512 channels.  Per core:
  - q/k/v projections of x[b] restricted to the group's 512 columns
  - causal attention for the 8 heads, computed in the transposed
    orientation S^T = [tk, tq] so softmax denominators come from a
    ones-column appended to V (AV matmul yields them for free) and no
    transposes are needed anywhere
  - partial output projection through the group's 512 rows of Wo
Host sums the two partials per batch and adds (bv @ Wo + bo): softmax
weights sum to one, so the v-bias passes through attention additively.

The QK stationary operand is zero-padded to a full [128,128] footprint
(kz holds each head's k tile in its 64 q-channel rows, zeros in the other
head's rows, so the full 128-partition q pair streams through) — without
this the PE activity monitor keeps the tensor engine clock-gated at
1.2 GHz for the whole attention phase.
"""

import numpy as np
import ml_dtypes

import concourse.bass as bass
import concourse.mybir as mybir
from concourse import bacc, tile
from concourse.bass_utils import run_bass_kernel_spmd

B, T, C, H = 4, 2048, 1024, 16
HD = C // H          # 64
G = 2                # head groups (cores per batch)
HG = H // G          # 8 heads per group
CG = C // G          # 512 channels per group
CGP = CG // 128      # 4 c_out tiles per group
P = 128
W = 512              # free-dim window (one PSUM bank of f32)
NW = T // W          # 4 windows
NTT = T // P         # 16 t tiles
NCI = C // P         # 8 c_in chunks
VS = HD + 1          # 65: v plus ones column

_cached_nc = None


def _build():
    f32 = mybir.dt.float32
    f32r = mybir.dt.float32r
    bf16 = mybir.dt.bfloat16
    AF = mybir.ActivationFunctionType
    nc = bacc.Bacc("TRN2", target_bir_lowering=False, debug=False, num_devices=8)

    xt_d = nc.dram_tensor("xt", [C, T], bf16, kind="ExternalInput")
    wq_d = nc.dram_tensor("wq", [C, CG], bf16, kind="ExternalInput")
    wk_d = nc.dram_tensor("wk", [C, CG], bf16, kind="ExternalInput")
    wv_d = nc.dram_tensor("wv", [C, CG], bf16, kind="ExternalInput")
    wo_d = nc.dram_tensor("wo", [CG, C], f32r, kind="ExternalInput")
    bq_d = nc.dram_tensor("bq", [P, CGP], f32, kind="ExternalInput")
    bk_d = nc.dram_tensor("bk", [P, CGP], f32, kind="ExternalInput")
    mask_d = nc.dram_tensor("mask", [P, P], f32, kind="ExternalInput")
    mask2_d = nc.dram_tensor("mask2", [P, 2 * P], f32, kind="ExternalInput")
    out_d = nc.dram_tensor("outp", [C, T], f32, kind="ExternalOutput")

    mm = lambda out, lhsT, rhs, start, stop: nc.tensor.matmul(
        out, lhsT, rhs, start=start, stop=stop)

    with tile.TileContext(nc) as tc:
        with tc.tile_pool(name="pers", bufs=1) as pers:
            qT = pers.tile([P, CGP, T], bf16)        # q^T: [c_out, t]
            # kz[:, j, a, tk]: head h=2j+a k-tile in its own 64 rows, 0 else
            kz = pers.tile([P, CGP, 2, T], bf16)
            vp = pers.tile([P, NTT, HG * VS], bf16)  # v rows + ones col/head
            wo_sb = pers.tile([P, CGP, C], f32r)
            mask_sb = pers.tile([P, P], f32)
            mask2_sb = pers.tile([P, 2 * P], f32)
            bq_sb = pers.tile([P, CGP], f32)
            bk_sb = pers.tile([P, CGP], f32)
            ones_sb = pers.tile([P, HG], f32)

            nc.sync.dma_start(out=wo_sb, in_=wo_d.ap().rearrange("(c p) n -> p c n", p=P))
            nc.sync.dma_start(out=mask_sb, in_=mask_d.ap())
            nc.sync.dma_start(out=mask2_sb, in_=mask2_d.ap())
            nc.sync.dma_start(out=bq_sb, in_=bq_d.ap())
            nc.sync.dma_start(out=bk_sb, in_=bk_d.ap())
            nc.vector.memset(ones_sb, 1.0)
            # zero halves of kz that the k-projection copies never touch
            nc.vector.memset(kz[0:HD, :, 1, :], 0.0)
            nc.vector.memset(kz[HD:P, :, 0, :], 0.0)

            # ---- phase B: projections, single pass over all 8 c_in chunks ----
            with (
                tc.tile_pool(name="xchunk", bufs=NCI) as xpool,
                tc.tile_pool(name="wchunk", bufs=1) as wpool,
                tc.tile_pool(name="psum_b", bufs=4, space="PSUM") as psum_b,
            ):
                wh = wpool.tile([P, 3, NCI, CG], bf16)
                for wi, wd in enumerate((wq_d, wk_d, wv_d)):
                    nc.sync.dma_start(
                        out=wh[:, wi, :, :],
                        in_=wd.ap().rearrange("(c p) n -> p c n", p=P),
                    )
                xc = []
                for ci in range(NCI):
                    t_ = xpool.tile([P, T], bf16, tag="xc")
                    # two half-tile DMAs so the first matmuls start sooner
                    nc.sync.dma_start(out=t_[:, 0:T // 2],
                                      in_=xt_d.ap()[ci * P:(ci + 1) * P, 0:T // 2])
                    nc.sync.dma_start(out=t_[:, T // 2:],
                                      in_=xt_d.ap()[ci * P:(ci + 1) * P, T // 2:])
                    xc.append(t_)

                def emit_qk(j, w):
                    ws = slice(w * W, (w + 1) * W)
                    psq = psum_b.tile([P, W], f32, tag="psb")
                    for i in range(NCI):
                        mm(psq, wh[:, 0, i, j * P:(j + 1) * P],
                           xc[i][:, ws], start=(i == 0), stop=(i == NCI - 1))
                    nc.scalar.activation(qT[:, j, ws], psq,
                                         AF.Identity, bias=bq_sb[:, j:j + 1])
                    psk = psum_b.tile([P, W], f32, tag="psb")
                    for i in range(NCI):
                        mm(psk, wh[:, 1, i, j * P:(j + 1) * P],
                           xc[i][:, ws], start=(i == 0), stop=(i == NCI - 1))
                    for a in range(2):
                        rows = slice(a * HD, (a + 1) * HD)
                        nc.scalar.activation(kz[rows, j, a, ws], psk[rows, :],
                                             AF.Identity, bias=bk_sb[rows, j:j + 1])

                def emit_v(it):
                    # v in row layout [t, c_out], strided into vp per head
                    psv = psum_b.tile([P, CG], f32, tag="psb")
                    for i in range(NCI):
                        mm(psv, xc[i][:, it * P:(it + 1) * P], wh[:, 2, i, :],
                           start=(i == 0), stop=(i == NCI - 1))
                    v_view = vp[:, it, :].rearrange("p (h x) -> p h x", x=VS)[:, :, 0:HD]
                    nc.scalar.copy(v_view, psv.rearrange("p (h x) -> p h x", x=HD))

                # window-major so early attention windows can start while the
                # projection tail still runs
                for w in range(NW):
                    for it in range(4 * w, 4 * w + 4):
                        emit_v(it)
                    for j in range(CGP):
                        emit_qk(j, w)
            for it in range(NTT):
                ones_view = vp[:, it, :].rearrange("p (h x) -> p h x", x=VS)[:, :, HD:VS]
                nc.vector.tensor_copy(
                    ones_view, ones_sb.rearrange("p (h x) -> p h x", x=1))
            # ---- phase C+D: attention with interleaved output projection ----
            with (
                tc.tile_pool(name="attn", bufs=1) as attn,
                tc.tile_pool(name="psum_mm", bufs=2, space="PSUM") as psum_mm,
                tc.tile_pool(name="pt", bufs=6) as ptpool,
                tc.tile_pool(name="dn", bufs=2) as dnpool,
                tc.tile_pool(name="rb", bufs=2) as rbpool,
                tc.tile_pool(name="psum_av", bufs=2, space="PSUM") as psum_av,
                tc.tile_pool(name="osb", bufs=3) as opool,
                tc.tile_pool(name="psum_o", bufs=2, space="PSUM") as psum_o,
            ):
                yT = attn.tile([P, CGP, T], f32r)
                escale = 1.0 / float(np.sqrt(HD))
                for w in range(NW):
                    tq0 = w * W
                    ntk = (w + 1) * (W // P)
                    for h in range(HG):
                        j, a = h // 2, h % 2
                        ps_av = psum_av.tile([VS, W], f32, tag="av")
                        # tk blocks go in pairs sharing one psum pair-tile and
                        # one exp; a diagonal second block is extended to the
                        # pair's region and cleaned up by mask2 (128 zero cols
                        # + 128 triangular cols)
                        for x in range(ntk // 2):
                            i0, i1 = 2 * x, 2 * x + 1
                            vs0 = max(tq0, i0 * P)
                            n0 = tq0 + W - vs0
                            ps_s = psum_mm.tile([P, 2, W], f32, tag="ps")
                            pt = ptpool.tile([P, 2, W], bf16, tag="pt")
                            mm(ps_s[:, 0, W - n0:], kz[:, j, a, i0 * P:(i0 + 1) * P],
                               qT[:, j, vs0:vs0 + n0], start=True, stop=True)
                            mm(ps_s[:, 1, W - n0:], kz[:, j, a, i1 * P:(i1 + 1) * P],
                               qT[:, j, vs0:vs0 + n0], start=True, stop=True)
                            nc.scalar.activation(pt[:, :, W - n0:], ps_s[:, :, W - n0:],
                                                 AF.Exp, scale=escale)
                            if i0 * P >= tq0:  # diagonal pair
                                nc.vector.tensor_mul(
                                    pt[:, 0, W - n0:W - n0 + P],
                                    pt[:, 0, W - n0:W - n0 + P], mask_sb)
                                nc.vector.tensor_mul(
                                    pt[:, 1, W - n0:W - n0 + 2 * P],
                                    pt[:, 1, W - n0:W - n0 + 2 * P], mask2_sb)
                            mm(ps_av[:, vs0 - tq0:], vp[:, i0, h * VS:(h + 1) * VS],
                               pt[:, 0, W - n0:], start=(i0 == 0), stop=False)
                            mm(ps_av[:, vs0 - tq0:], vp[:, i1, h * VS:(h + 1) * VS],
                               pt[:, 1, W - n0:], start=False, stop=(i1 == ntk - 1))
                        dn = dnpool.tile([1, W], f32, tag="dn")
                        nc.vector.tensor_copy(dn, ps_av[HD:VS, :])
                        rb = rbpool.tile([HD, W], f32, tag="rb")
                        nc.gpsimd.partition_broadcast(rb, dn)
                        nc.vector.reciprocal_approx_fast(out=rb, in_=rb)
                        nc.vector.tensor_mul(
                            yT[a * HD:(a + 1) * HD, j, tq0:tq0 + W],
                            ps_av[0:HD, :], rb)
                    # output projection for this window
                    for m in range(C // P):
                        po = psum_o.tile([P, W], f32, tag="po")
                        for i in range(CGP):
                            mm(po, wo_sb[:, i, m * P:(m + 1) * P],
                               yT[:, i, tq0:tq0 + W],
                               start=(i == 0), stop=(i == CGP - 1))
                        ot = opool.tile([P, W], f32, tag="ot")
                        nc.vector.tensor_copy(ot, po)
                        nc.sync.dma_start(
                            out=out_d.ap()[m * P:(m + 1) * P, tq0:tq0 + W],
                            in_=ot)

    nc.compile()
    return nc


def get_nc():
    global _cached_nc
    if _cached_nc is None:
        _cached_nc = _build()
    return _cached_nc


def make_in_maps(x, Wq, bq, Wk, bk, Wv, bv, Wo, bo):
    x = np.asarray(x, np.float32)
    mask = np.triu(np.ones((P, P), np.float32))
    mask2 = np.concatenate([np.zeros((P, P), np.float32), mask], axis=1)
    in_maps = []
    for c in range(8):
        b, g = c // 2, c % 2
        cs = slice(g * CG, (g + 1) * CG)
        in_maps.append({
            "xt": np.ascontiguousarray(x[b].T.astype(ml_dtypes.bfloat16)),
            "wq": np.ascontiguousarray(
                np.asarray(Wq, np.float32)[:, cs].astype(ml_dtypes.bfloat16)),
            "wk": np.ascontiguousarray(
                np.asarray(Wk, np.float32)[:, cs].astype(ml_dtypes.bfloat16)),
            "wv": np.ascontiguousarray(
                np.asarray(Wv, np.float32)[:, cs].astype(ml_dtypes.bfloat16)),
            "wo": np.ascontiguousarray(np.asarray(Wo, np.float32)[cs, :]),
            "bq": np.ascontiguousarray(
                np.asarray(bq, np.float32)[cs].reshape(CGP, P).T),
            "bk": np.ascontiguousarray(
                np.asarray(bk, np.float32)[cs].reshape(CGP, P).T),
            "mask": mask,
            "mask2": mask2,
        })
    return in_maps


def combine(results, Wv, bv, Wo, bo):
    const = (np.asarray(bv, np.float32) @ np.asarray(Wo, np.float32)
             + np.asarray(bo, np.float32))
    out = np.empty((B, T, C), np.float32)
    for b in range(B):
        acc = results[2 * b]["outp"] + results[2 * b + 1]["outp"]
        out[b] = acc.T + const[None, :]
    return out


def kernel(x, Wq, bq, Wk, bk, Wv, bv, Wo, bo):
    nc = get_nc()
    in_maps = make_in_maps(x, Wq, bq, Wk, bk, Wv, bv, Wo, bo)
    res = run_bass_kernel_spmd(nc, in_maps, core_ids=list(range(8)))
    return combine(res.results, Wv, bv, Wo, bo)


# revision 23
# speedup vs baseline: 1.1764x; 1.0108x over previous
"""Causal self-attention on 8 trn2 NeuronCores.

Sharding: core c -> (batch b = c//2, head-group g = c%2).  Each head-group
is 8 heads = 512 channels.  Per core:
  - q/k/v projections of x[b] restricted to the group's 512 columns
  - causal attention for the 8 heads, computed in the transposed
    orientation S^T = [tk, tq] so softmax denominators come from a
    ones-column appended to V (AV matmul yields them for free) and no
    transposes are needed anywhere
  - partial output projection through the group's 512 rows of Wo
Host sums the two partials per batch and adds (bv @ Wo + bo): softmax
weights sum to one, so the v-bias passes through attention additively.

The QK stationary operand is zero-padded to a full [128,128] footprint
(kz holds each head's k tile in its 64 q-channel rows, zeros in the other
head's rows, so the full 128-partition q pair streams through) — without
this the PE activity monitor keeps the tensor engine clock-gated at
1.2 GHz for the whole attention phase.
"""

import numpy as np
import ml_dtypes

import concourse.bass as bass
import concourse.mybir as mybir
from concourse import bacc, tile
from concourse.bass_utils import run_bass_kernel_spmd

B, T, C, H = 4, 2048, 1024, 16
HD = C // H          # 64
G = 2                # head groups (cores per batch)
HG = H // G          # 8 heads per group
CG = C // G          # 512 channels per group
CGP = CG // 128      # 4 c_out tiles per group
P = 128
W = 512              # free-dim window (one PSUM bank of f32)
NW = T // W          # 4 windows
NTT = T // P         # 16 t tiles
NCI = C // P         # 8 c_in chunks
VS = HD + 1          # 65: v plus ones column

_cached_nc = None


def _build():
    f32 = mybir.dt.float32
    f32r = mybir.dt.float32r
    bf16 = mybir.dt.bfloat16
    AF = mybir.ActivationFunctionType
    nc = bacc.Bacc("TRN2", target_bir_lowering=False, debug=False, num_devices=8)

    xt_d = nc.dram_tensor("xt", [C, T], bf16, kind="ExternalInput")
    wq_d = nc.dram_tensor("wq", [C, CG], bf16, kind="ExternalInput")
    wk_d = nc.dram_tensor("wk", [C, CG], bf16, kind="ExternalInput")
    wv_d = nc.dram_tensor("wv", [C, CG], bf16, kind="ExternalInput")
    wo_d = nc.dram_tensor("wo", [CG, C], f32r, kind="ExternalInput")
    bq_d = nc.dram_tensor("bq", [P, CGP], f32, kind="ExternalInput")
    bk_d = nc.dram_tensor("bk", [P, CGP], f32, kind="ExternalInput")
    mask_d = nc.dram_tensor("mask", [P, P], f32, kind="ExternalInput")
    mask2_d = nc.dram_tensor("mask2", [P, 2 * P], f32, kind="ExternalInput")
    out_d = nc.dram_tensor("outp", [C, T], f32, kind="ExternalOutput")

    mm = lambda out, lhsT, rhs, start, stop: nc.tensor.matmul(
        out, lhsT, rhs, start=start, stop=stop)

    with tile.TileContext(nc) as tc:
        with tc.tile_pool(name="pers", bufs=1) as pers:
            qT = pers.tile([P, CGP, T], bf16)        # q^T: [c_out, t]
            # kz[:, j, a, tk]: head h=2j+a k-tile in its own 64 rows, 0 else
            kz = pers.tile([P, CGP, 2, T], bf16)
            vp = pers.tile([P, NTT, HG * VS], bf16)  # v rows + ones col/head
            wo_sb = pers.tile([P, CGP, C], f32r)
            mask_sb = pers.tile([P, P], f32)
            mask2_sb = pers.tile([P, 2 * P], f32)
            bq_sb = pers.tile([P, CGP], f32)
            bk_sb = pers.tile([P, CGP], f32)
            ones_sb = pers.tile([P, HG], f32)

            nc.sync.dma_start(out=bq_sb, in_=bq_d.ap())
            nc.sync.dma_start(out=bk_sb, in_=bk_d.ap())
            nc.vector.memset(ones_sb, 1.0)
            # zero halves of kz that the k-projection copies never touch
            nc.vector.memset(kz[0:HD, :, 1, :], 0.0)
            nc.vector.memset(kz[HD:P, :, 0, :], 0.0)

            # ---- phase B: projections, single pass over all 8 c_in chunks ----
            with (
                tc.tile_pool(name="xchunk", bufs=NCI) as xpool,
                tc.tile_pool(name="wchunk", bufs=1) as wpool,
                tc.tile_pool(name="psum_b", bufs=4, space="PSUM") as psum_b,
            ):
                wh = wpool.tile([P, 3, NCI, CG], bf16)
                for wi, wd in enumerate((wq_d, wk_d, wv_d)):
                    nc.sync.dma_start(
                        out=wh[:, wi, :, :],
                        in_=wd.ap().rearrange("(c p) n -> p c n", p=P),
                    )
                xc = []
                for ci in range(NCI):
                    t_ = xpool.tile([P, T], bf16, tag="xc")
                    # two half-tile DMAs so the first matmuls start sooner
                    nc.sync.dma_start(out=t_[:, 0:T // 2],
                                      in_=xt_d.ap()[ci * P:(ci + 1) * P, 0:T // 2])
                    nc.sync.dma_start(out=t_[:, T // 2:],
                                      in_=xt_d.ap()[ci * P:(ci + 1) * P, T // 2:])
                    xc.append(t_)
                if True:
                    nc.sync.dma_start(out=mask_sb, in_=mask_d.ap())
                    nc.sync.dma_start(out=mask2_sb, in_=mask2_d.ap())
                    nc.sync.dma_start(
                        out=wo_sb, in_=wo_d.ap().rearrange("(c p) n -> p c n", p=P))

                def emit_qk(j, w):
                    ws = slice(w * W, (w + 1) * W)
                    psq = psum_b.tile([P, W], f32, tag="psb")
                    for i in range(NCI):
                        mm(psq, wh[:, 0, i, j * P:(j + 1) * P],
                           xc[i][:, ws], start=(i == 0), stop=(i == NCI - 1))
                    nc.scalar.activation(qT[:, j, ws], psq,
                                         AF.Identity, bias=bq_sb[:, j:j + 1])
                    psk = psum_b.tile([P, W], f32, tag="psb")
                    for i in range(NCI):
                        mm(psk, wh[:, 1, i, j * P:(j + 1) * P],
                           xc[i][:, ws], start=(i == 0), stop=(i == NCI - 1))
                    for a in range(2):
                        rows = slice(a * HD, (a + 1) * HD)
                        nc.scalar.activation(kz[rows, j, a, ws], psk[rows, :],
                                             AF.Identity, bias=bk_sb[rows, j:j + 1])

                def emit_v(it):
                    # v in row layout [t, c_out], strided into vp per head
                    psv = psum_b.tile([P, CG], f32, tag="psb")
                    for i in range(NCI):
                        mm(psv, xc[i][:, it * P:(it + 1) * P], wh[:, 2, i, :],
                           start=(i == 0), stop=(i == NCI - 1))
                    v_view = vp[:, it, :].rearrange("p (h x) -> p h x", x=VS)[:, :, 0:HD]
                    nc.scalar.copy(v_view, psv.rearrange("p (h x) -> p h x", x=HD))

                # window-major so early attention windows can start while the
                # projection tail still runs
                for w in range(NW):
                    for it in range(4 * w, 4 * w + 4):
                        emit_v(it)
                    for j in range(CGP):
                        emit_qk(j, w)
            for it in range(NTT):
                ones_view = vp[:, it, :].rearrange("p (h x) -> p h x", x=VS)[:, :, HD:VS]
                nc.vector.tensor_copy(
                    ones_view, ones_sb.rearrange("p (h x) -> p h x", x=1))
            # ---- phase C+D: attention with interleaved output projection ----
            with (
                tc.tile_pool(name="attn", bufs=1) as attn,
                tc.tile_pool(name="psum_mm", bufs=2, space="PSUM") as psum_mm,
                tc.tile_pool(name="pt", bufs=6) as ptpool,
                tc.tile_pool(name="dn", bufs=2) as dnpool,
                tc.tile_pool(name="rb", bufs=2) as rbpool,
                tc.tile_pool(name="psum_av", bufs=2, space="PSUM") as psum_av,
                tc.tile_pool(name="osb", bufs=3) as opool,
                tc.tile_pool(name="psum_o", bufs=2, space="PSUM") as psum_o,
            ):
                yT = attn.tile([P, CGP, T], f32r)
                escale = 1.0 / float(np.sqrt(HD))
                for w in range(NW):
                    tq0 = w * W
                    ntk = (w + 1) * (W // P)
                    for h in range(HG):
                        j, a = h // 2, h % 2
                        ps_av = psum_av.tile([VS, W], f32, tag="av")
                        # tk blocks go in pairs sharing one psum pair-tile and
                        # one exp; a diagonal second block is extended to the
                        # pair's region and cleaned up by mask2 (128 zero cols
                        # + 128 triangular cols)
                        for x in range(ntk // 2):
                            i0, i1 = 2 * x, 2 * x + 1
                            vs0 = max(tq0, i0 * P)
                            n0 = tq0 + W - vs0
                            ps_s = psum_mm.tile([P, 2, W], f32, tag="ps")
                            pt = ptpool.tile([P, 2, W], bf16, tag="pt")
                            mm(ps_s[:, 0, W - n0:], kz[:, j, a, i0 * P:(i0 + 1) * P],
                               qT[:, j, vs0:vs0 + n0], start=True, stop=True)
                            mm(ps_s[:, 1, W - n0:], kz[:, j, a, i1 * P:(i1 + 1) * P],
                               qT[:, j, vs0:vs0 + n0], start=True, stop=True)
                            nc.scalar.activation(pt[:, :, W - n0:], ps_s[:, :, W - n0:],
                                                 AF.Exp, scale=escale)
                            if i0 * P >= tq0:  # diagonal pair
                                nc.vector.tensor_mul(
                                    pt[:, 0, W - n0:W - n0 + P],
                                    pt[:, 0, W - n0:W - n0 + P], mask_sb)
                                nc.vector.tensor_mul(
                                    pt[:, 1, W - n0:W - n0 + 2 * P],
                                    pt[:, 1, W - n0:W - n0 + 2 * P], mask2_sb)
                            mm(ps_av[:, vs0 - tq0:], vp[:, i0, h * VS:(h + 1) * VS],
                               pt[:, 0, W - n0:], start=(i0 == 0), stop=False)
                            mm(ps_av[:, vs0 - tq0:], vp[:, i1, h * VS:(h + 1) * VS],
                               pt[:, 1, W - n0:], start=False, stop=(i1 == ntk - 1))
                        dn = dnpool.tile([1, W], f32, tag="dn")
                        nc.vector.tensor_copy(dn, ps_av[HD:VS, :])
                        rb = rbpool.tile([HD, W], f32, tag="rb")
                        nc.gpsimd.partition_broadcast(rb, dn)
                        nc.vector.reciprocal_approx_fast(out=rb, in_=rb)
                        nc.vector.tensor_mul(
                            yT[a * HD:(a + 1) * HD, j, tq0:tq0 + W],
                            ps_av[0:HD, :], rb)
                    # output projection for this window
                    for m in range(C // P):
                        po = psum_o.tile([P, W], f32, tag="po")
                        for i in range(CGP):
                            mm(po, wo_sb[:, i, m * P:(m + 1) * P],
                               yT[:, i, tq0:tq0 + W],
                               start=(i == 0), stop=(i == CGP - 1))
                        ot = opool.tile([P, W], f32, tag="ot")
                        nc.vector.tensor_copy(ot, po)
                        nc.sync.dma_start(
                            out=out_d.ap()[m * P:(m + 1) * P, tq0:tq0 + W],
                            in_=ot)

    nc.compile()
    return nc


def get_nc():
    global _cached_nc
    if _cached_nc is None:
        _cached_nc = _build()
    return _cached_nc


def make_in_maps(x, Wq, bq, Wk, bk, Wv, bv, Wo, bo):
    x = np.asarray(x, np.float32)
    mask = np.triu(np.ones((P, P), np.float32))
    mask2 = np.concatenate([np.zeros((P, P), np.float32), mask], axis=1)
    in_maps = []
    for c in range(8):
        b, g = c // 2, c % 2
        cs = slice(g * CG, (g + 1) * CG)
        in_maps.append({
            "xt": np.ascontiguousarray(x[b].T.astype(ml_dtypes.bfloat16)),
            "wq": np.ascontiguousarray(
                np.asarray(Wq, np.float32)[:, cs].astype(ml_dtypes.bfloat16)),
            "wk": np.ascontiguousarray(
                np.asarray(Wk, np.float32)[:, cs].astype(ml_dtypes.bfloat16)),
            "wv": np.ascontiguousarray(
                np.asarray(Wv, np.float32)[:, cs].astype(ml_dtypes.bfloat16)),
            "wo": np.ascontiguousarray(np.asarray(Wo, np.float32)[cs, :]),
            "bq": np.ascontiguousarray(
                np.asarray(bq, np.float32)[cs].reshape(CGP, P).T),
            "bk": np.ascontiguousarray(
                np.asarray(bk, np.float32)[cs].reshape(CGP, P).T),
            "mask": mask,
            "mask2": mask2,
        })
    return in_maps


def combine(results, Wv, bv, Wo, bo):
    const = (np.asarray(bv, np.float32) @ np.asarray(Wo, np.float32)
             + np.asarray(bo, np.float32))
    out = np.empty((B, T, C), np.float32)
    for b in range(B):
        acc = results[2 * b]["outp"] + results[2 * b + 1]["outp"]
        out[b] = acc.T + const[None, :]
    return out


def kernel(x, Wq, bq, Wk, bk, Wv, bv, Wo, bo):
    nc = get_nc()
    in_maps = make_in_maps(x, Wq, bq, Wk, bk, Wv, bv, Wo, bo)
    res = run_bass_kernel_spmd(nc, in_maps, core_ids=list(range(8)))
    return combine(res.results, Wv, bv, Wo, bo)


# revision 24
# speedup vs baseline: 1.1792x; 1.0024x over previous
"""Causal self-attention on 8 trn2 NeuronCores.

Sharding: core c -> (batch b = c//2, head-group g = c%2).  Each head-group
is 8 heads = 512 channels.  Per core:
  - q/k/v projections of x[b] restricted to the group's 512 columns
  - causal attention for the 8 heads, computed in the transposed
    orientation S^T = [tk, tq] so softmax denominators come from a
    ones-column appended to V (AV matmul yields them for free) and no
    transposes are needed anywhere
  - partial output projection through the group's 512 rows of Wo
Host sums the two partials per batch and adds (bv @ Wo + bo): softmax
weights sum to one, so the v-bias passes through attention additively.

The QK stationary operand is zero-padded to a full [128,128] footprint
(kz holds each head's k tile in its 64 q-channel rows, zeros in the other
head's rows, so the full 128-partition q pair streams through) — without
this the PE activity monitor keeps the tensor engine clock-gated at
1.2 GHz for the whole attention phase.
"""

import numpy as np
import ml_dtypes

import concourse.bass as bass
import concourse.mybir as mybir
from concourse import bacc, tile
from concourse.bass_utils import run_bass_kernel_spmd

B, T, C, H = 4, 2048, 1024, 16
HD = C // H          # 64
G = 2                # head groups (cores per batch)
HG = H // G          # 8 heads per group
CG = C // G          # 512 channels per group
CGP = CG // 128      # 4 c_out tiles per group
P = 128
W = 512              # free-dim window (one PSUM bank of f32)
NW = T // W          # 4 windows
NTT = T // P         # 16 t tiles
NCI = C // P         # 8 c_in chunks
VS = HD + 1          # 65: v plus ones column

_cached_nc = None


def _build():
    f32 = mybir.dt.float32
    f32r = mybir.dt.float32r
    bf16 = mybir.dt.bfloat16
    AF = mybir.ActivationFunctionType
    nc = bacc.Bacc("TRN2", target_bir_lowering=False, debug=False, num_devices=8)

    xt_d = nc.dram_tensor("xt", [C, T], bf16, kind="ExternalInput")
    wq_d = nc.dram_tensor("wq", [C, CG], bf16, kind="ExternalInput")
    wk_d = nc.dram_tensor("wk", [C, CG], bf16, kind="ExternalInput")
    wv_d = nc.dram_tensor("wv", [C, CG], bf16, kind="ExternalInput")
    wo_d = nc.dram_tensor("wo", [CG, C], f32r, kind="ExternalInput")
    bq_d = nc.dram_tensor("bq", [P, CGP], f32, kind="ExternalInput")
    bk_d = nc.dram_tensor("bk", [P, CGP], f32, kind="ExternalInput")
    mask_d = nc.dram_tensor("mask", [P, P], f32, kind="ExternalInput")
    mask2_d = nc.dram_tensor("mask2", [P, 2 * P], f32, kind="ExternalInput")
    out_d = nc.dram_tensor("outp", [C, T], f32, kind="ExternalOutput")

    mm = lambda out, lhsT, rhs, start, stop: nc.tensor.matmul(
        out, lhsT, rhs, start=start, stop=stop)

    with tile.TileContext(nc) as tc:
        with tc.tile_pool(name="pers", bufs=1) as pers:
            qT = pers.tile([P, CGP, T], bf16)        # q^T: [c_out, t]
            # kz[:, j, a, tk]: head h=2j+a k-tile in its own 64 rows, 0 else
            kz = pers.tile([P, CGP, 2, T], bf16)
            vp = pers.tile([P, NTT, HG * VS], bf16)  # v rows + ones col/head
            wo_sb = pers.tile([P, CGP, C], f32r)
            mask_sb = pers.tile([P, P], f32)
            mask2_sb = pers.tile([P, 2 * P], f32)
            bq_sb = pers.tile([P, CGP], f32)
            bk_sb = pers.tile([P, CGP], f32)
            ones_sb = pers.tile([P, HG], f32)

            nc.sync.dma_start(out=bq_sb, in_=bq_d.ap())
            nc.sync.dma_start(out=bk_sb, in_=bk_d.ap())
            nc.vector.memset(ones_sb, 1.0)
            # zero halves of kz that the k-projection copies never touch
            nc.vector.memset(kz[0:HD, :, 1, :], 0.0)
            nc.vector.memset(kz[HD:P, :, 0, :], 0.0)

            # ---- phase B: projections, single pass over all 8 c_in chunks ----
            with (
                tc.tile_pool(name="xchunk", bufs=NCI) as xpool,
                tc.tile_pool(name="wchunk", bufs=1) as wpool,
                tc.tile_pool(name="psum_b", bufs=4, space="PSUM") as psum_b,
            ):
                wh = wpool.tile([P, 3, NCI, CG], bf16)
                for wi, wd in enumerate((wq_d, wk_d, wv_d)):
                    nc.sync.dma_start(
                        out=wh[:, wi, :, :],
                        in_=wd.ap().rearrange("(c p) n -> p c n", p=P),
                    )
                xc = []
                for ci in range(NCI):
                    t_ = xpool.tile([P, T], bf16, tag="xc")
                    # two half-tile DMAs so the first matmuls start sooner
                    nc.sync.dma_start(out=t_[:, 0:T // 2],
                                      in_=xt_d.ap()[ci * P:(ci + 1) * P, 0:T // 2])
                    nc.sync.dma_start(out=t_[:, T // 2:],
                                      in_=xt_d.ap()[ci * P:(ci + 1) * P, T // 2:])
                    xc.append(t_)
                if True:
                    nc.sync.dma_start(out=mask_sb, in_=mask_d.ap())
                    nc.sync.dma_start(out=mask2_sb, in_=mask2_d.ap())
                    nc.sync.dma_start(
                        out=wo_sb, in_=wo_d.ap().rearrange("(c p) n -> p c n", p=P))

                def emit_qk(j, w):
                    ws = slice(w * W, (w + 1) * W)
                    psq = psum_b.tile([P, W], f32, tag="psb")
                    for i in range(NCI):
                        mm(psq, wh[:, 0, i, j * P:(j + 1) * P],
                           xc[i][:, ws], start=(i == 0), stop=(i == NCI - 1))
                    nc.vector.tensor_scalar_add(qT[:, j, ws], psq,
                                                bq_sb[:, j:j + 1])
                    psk = psum_b.tile([P, W], f32, tag="psb")
                    for i in range(NCI):
                        mm(psk, wh[:, 1, i, j * P:(j + 1) * P],
                           xc[i][:, ws], start=(i == 0), stop=(i == NCI - 1))
                    for a in range(2):
                        rows = slice(a * HD, (a + 1) * HD)
                        nc.vector.tensor_scalar_add(kz[rows, j, a, ws],
                                                    psk[rows, :],
                                                    bk_sb[rows, j:j + 1])

                def emit_v(it):
                    # v in row layout [t, c_out], strided into vp per head
                    psv = psum_b.tile([P, CG], f32, tag="psb")
                    for i in range(NCI):
                        mm(psv, xc[i][:, it * P:(it + 1) * P], wh[:, 2, i, :],
                           start=(i == 0), stop=(i == NCI - 1))
                    v_view = vp[:, it, :].rearrange("p (h x) -> p h x", x=VS)[:, :, 0:HD]
                    nc.vector.tensor_copy(v_view, psv.rearrange("p (h x) -> p h x", x=HD))

                # window-major so early attention windows can start while the
                # projection tail still runs
                for w in range(NW):
                    for it in range(4 * w, 4 * w + 4):
                        emit_v(it)
                    for j in range(CGP):
                        emit_qk(j, w)
            for it in range(NTT):
                ones_view = vp[:, it, :].rearrange("p (h x) -> p h x", x=VS)[:, :, HD:VS]
                nc.vector.tensor_copy(
                    ones_view, ones_sb.rearrange("p (h x) -> p h x", x=1))
            # ---- phase C+D: attention with interleaved output projection ----
            with (
                tc.tile_pool(name="attn", bufs=1) as attn,
                tc.tile_pool(name="psum_mm", bufs=2, space="PSUM") as psum_mm,
                tc.tile_pool(name="pt", bufs=8) as ptpool,
                tc.tile_pool(name="dn", bufs=4) as dnpool,
                tc.tile_pool(name="rb", bufs=4) as rbpool,
                tc.tile_pool(name="psum_av", bufs=2, space="PSUM") as psum_av,
                tc.tile_pool(name="osb", bufs=3) as opool,
                tc.tile_pool(name="psum_o", bufs=2, space="PSUM") as psum_o,
            ):
                yT = attn.tile([P, CGP, T], f32r)
                escale = 1.0 / float(np.sqrt(HD))
                for w in range(NW):
                    tq0 = w * W
                    ntk = (w + 1) * (W // P)
                    for h in range(HG):
                        j, a = h // 2, h % 2
                        ps_av = psum_av.tile([VS, W], f32, tag="av")
                        # tk blocks go in pairs sharing one psum pair-tile and
                        # one exp; a diagonal second block is extended to the
                        # pair's region and cleaned up by mask2 (128 zero cols
                        # + 128 triangular cols)
                        for x in range(ntk // 2):
                            i0, i1 = 2 * x, 2 * x + 1
                            vs0 = max(tq0, i0 * P)
                            n0 = tq0 + W - vs0
                            ps_s = psum_mm.tile([P, 2, W], f32, tag="ps")
                            pt = ptpool.tile([P, 2, W], bf16, tag="pt")
                            mm(ps_s[:, 0, W - n0:], kz[:, j, a, i0 * P:(i0 + 1) * P],
                               qT[:, j, vs0:vs0 + n0], start=True, stop=True)
                            mm(ps_s[:, 1, W - n0:], kz[:, j, a, i1 * P:(i1 + 1) * P],
                               qT[:, j, vs0:vs0 + n0], start=True, stop=True)
                            nc.scalar.activation(pt[:, :, W - n0:], ps_s[:, :, W - n0:],
                                                 AF.Exp, scale=escale)
                            if i0 * P >= tq0:  # diagonal pair
                                nc.vector.tensor_mul(
                                    pt[:, 0, W - n0:W - n0 + P],
                                    pt[:, 0, W - n0:W - n0 + P], mask_sb)
                                nc.vector.tensor_mul(
                                    pt[:, 1, W - n0:W - n0 + 2 * P],
                                    pt[:, 1, W - n0:W - n0 + 2 * P], mask2_sb)
                            mm(ps_av[:, vs0 - tq0:], vp[:, i0, h * VS:(h + 1) * VS],
                               pt[:, 0, W - n0:], start=(i0 == 0), stop=False)
                            mm(ps_av[:, vs0 - tq0:], vp[:, i1, h * VS:(h + 1) * VS],
                               pt[:, 1, W - n0:], start=False, stop=(i1 == ntk - 1))
                        dn = dnpool.tile([1, W], f32, tag="dn")
                        nc.vector.tensor_copy(dn, ps_av[HD:VS, :])
                        rb = rbpool.tile([HD, W], f32, tag="rb")
                        nc.gpsimd.partition_broadcast(rb, dn)
                        nc.vector.reciprocal_approx_fast(out=rb, in_=rb)
                        nc.vector.tensor_mul(
                            yT[a * HD:(a + 1) * HD, j, tq0:tq0 + W],
                            ps_av[0:HD, :], rb)
                    # output projection for this window
                    for m in range(C // P):
                        po = psum_o.tile([P, W], f32, tag="po")
                        for i in range(CGP):
                            mm(po, wo_sb[:, i, m * P:(m + 1) * P],
                               yT[:, i, tq0:tq0 + W],
                               start=(i == 0), stop=(i == CGP - 1))
                        ot = opool.tile([P, W], f32, tag="ot")
                        nc.vector.tensor_copy(ot, po)
                        nc.sync.dma_start(
                            out=out_d.ap()[m * P:(m + 1) * P, tq0:tq0 + W],
                            in_=ot)

    nc.compile()
    return nc


def get_nc():
    global _cached_nc
    if _cached_nc is None:
        _cached_nc = _build()
    return _cached_nc


def make_in_maps(x, Wq, bq, Wk, bk, Wv, bv, Wo, bo):
    x = np.asarray(x, np.float32)
    mask = np.triu(np.ones((P, P), np.float32))
    mask2 = np.concatenate([np.zeros((P, P), np.float32), mask], axis=1)
    in_maps = []
    for c in range(8):
        b, g = c // 2, c % 2
        cs = slice(g * CG, (g + 1) * CG)
        in_maps.append({
            "xt": np.ascontiguousarray(x[b].T.astype(ml_dtypes.bfloat16)),
            "wq": np.ascontiguousarray(
                np.asarray(Wq, np.float32)[:, cs].astype(ml_dtypes.bfloat16)),
            "wk": np.ascontiguousarray(
                np.asarray(Wk, np.float32)[:, cs].astype(ml_dtypes.bfloat16)),
            "wv": np.ascontiguousarray(
                np.asarray(Wv, np.float32)[:, cs].astype(ml_dtypes.bfloat16)),
            "wo": np.ascontiguousarray(np.asarray(Wo, np.float32)[cs, :]),
            "bq": np.ascontiguousarray(
                np.asarray(bq, np.float32)[cs].reshape(CGP, P).T),
            "bk": np.ascontiguousarray(
                np.asarray(bk, np.float32)[cs].reshape(CGP, P).T),
            "mask": mask,
            "mask2": mask2,
        })
    return in_maps


def combine(results, Wv, bv, Wo, bo):
    const = (np.asarray(bv, np.float32) @ np.asarray(Wo, np.float32)
             + np.asarray(bo, np.float32))
    out = np.empty((B, T, C), np.float32)
    for b in range(B):
        acc = results[2 * b]["outp"] + results[2 * b + 1]["outp"]
        out[b] = acc.T + const[None, :]
    return out


def kernel(x, Wq, bq, Wk, bk, Wv, bv, Wo, bo):
    nc = get_nc()
    in_maps = make_in_maps(x, Wq, bq, Wk, bk, Wv, bv, Wo, bo)
    res = run_bass_kernel_spmd(nc, in_maps, core_ids=list(range(8)))
    return combine(res.results, Wv, bv, Wo, bo)


# revision 27
# speedup vs baseline: 1.1829x; 1.0031x over previous
"""Causal self-attention on 8 trn2 NeuronCores.

Sharding: core c -> (batch b = c//2, head-group g = c%2).  Each head-group
is 8 heads = 512 channels.  Per core:
  - q/k/v projections of x[b] restricted to the group's 512 columns
  - causal attention for the 8 heads, computed in the transposed
    orientation S^T = [tk, tq] so softmax denominators come from a
    ones-column appended to V (AV matmul yields them for free) and no
    transposes are needed anywhere
  - partial output projection through the group's 512 rows of Wo
Host sums the two partials per batch and adds (bv @ Wo + bo): softmax
weights sum to one, so the v-bias passes through attention additively.

The QK stationary operand is zero-padded to a full [128,128] footprint
(kz holds each head's k tile in its 64 q-channel rows, zeros in the other
head's rows, so the full 128-partition q pair streams through) — without
this the PE activity monitor keeps the tensor engine clock-gated at
1.2 GHz for the whole attention phase.
"""

import numpy as np
import ml_dtypes

import concourse.bass as bass
import concourse.mybir as mybir
from concourse import bacc, tile
from concourse.bass_utils import run_bass_kernel_spmd

B, T, C, H = 4, 2048, 1024, 16
HD = C // H          # 64
G = 2                # head groups (cores per batch)
HG = H // G          # 8 heads per group
CG = C // G          # 512 channels per group
CGP = CG // 128      # 4 c_out tiles per group
P = 128
W = 512              # free-dim window (one PSUM bank of f32)
NW = T // W          # 4 windows
NTT = T // P         # 16 t tiles
NCI = C // P         # 8 c_in chunks
VS = HD + 1          # 65: v plus ones column

_cached_nc = None


def _build():
    f32 = mybir.dt.float32
    f32r = mybir.dt.float32r
    bf16 = mybir.dt.bfloat16
    AF = mybir.ActivationFunctionType
    nc = bacc.Bacc("TRN2", target_bir_lowering=False, debug=False, num_devices=8)

    xt_d = nc.dram_tensor("xt", [C, T], bf16, kind="ExternalInput")
    wq_d = nc.dram_tensor("wq", [C, CG], bf16, kind="ExternalInput")
    wk_d = nc.dram_tensor("wk", [C, CG], bf16, kind="ExternalInput")
    wv_d = nc.dram_tensor("wv", [C, CG], bf16, kind="ExternalInput")
    wo_d = nc.dram_tensor("wo", [CG, C], bf16, kind="ExternalInput")
    bq_d = nc.dram_tensor("bq", [P, CGP], f32, kind="ExternalInput")
    bk_d = nc.dram_tensor("bk", [P, CGP], f32, kind="ExternalInput")
    mask_d = nc.dram_tensor("mask", [P, P], f32, kind="ExternalInput")
    mask2_d = nc.dram_tensor("mask2", [P, 2 * P], f32, kind="ExternalInput")
    out_d = nc.dram_tensor("outp", [C, T], f32, kind="ExternalOutput")

    mm = lambda out, lhsT, rhs, start, stop: nc.tensor.matmul(
        out, lhsT, rhs, start=start, stop=stop)

    with tile.TileContext(nc) as tc:
        with tc.tile_pool(name="pers", bufs=1) as pers:
            qT = pers.tile([P, CGP, T], bf16)        # q^T: [c_out, t]
            # kz[:, j, a, tk]: head h=2j+a k-tile in its own 64 rows, 0 else
            kz = pers.tile([P, CGP, 2, T], bf16)
            vp = pers.tile([P, NTT, HG * VS], bf16)  # v rows + ones col/head
            wo_sb = pers.tile([P, CGP, C], bf16)
            mask_sb = pers.tile([P, P], f32)
            mask2_sb = pers.tile([P, 2 * P], f32)
            bq_sb = pers.tile([P, CGP], f32)
            bk_sb = pers.tile([P, CGP], f32)
            ones_sb = pers.tile([P, HG], f32)

            nc.sync.dma_start(out=bq_sb, in_=bq_d.ap())
            nc.sync.dma_start(out=bk_sb, in_=bk_d.ap())
            nc.vector.memset(ones_sb, 1.0)
            # zero halves of kz that the k-projection copies never touch
            nc.vector.memset(kz[0:HD, :, 1, :], 0.0)
            nc.vector.memset(kz[HD:P, :, 0, :], 0.0)

            # ---- phase B: projections, single pass over all 8 c_in chunks ----
            with (
                tc.tile_pool(name="xchunk", bufs=NCI) as xpool,
                tc.tile_pool(name="wchunk", bufs=1) as wpool,
                tc.tile_pool(name="psum_b", bufs=4, space="PSUM") as psum_b,
            ):
                wh = wpool.tile([P, 3, NCI, CG], bf16)
                for wi, wd in ((2, wv_d), (0, wq_d), (1, wk_d)):
                    nc.sync.dma_start(
                        out=wh[:, wi, :, :],
                        in_=wd.ap().rearrange("(c p) n -> p c n", p=P),
                    )
                xc = []
                for ci in range(NCI):
                    t_ = xpool.tile([P, T], bf16, tag="xc")
                    # two half-tile DMAs so the first matmuls start sooner
                    nc.sync.dma_start(out=t_[:, 0:T // 2],
                                      in_=xt_d.ap()[ci * P:(ci + 1) * P, 0:T // 2])
                    nc.sync.dma_start(out=t_[:, T // 2:],
                                      in_=xt_d.ap()[ci * P:(ci + 1) * P, T // 2:])
                    xc.append(t_)
                if True:
                    nc.sync.dma_start(out=mask_sb, in_=mask_d.ap())
                    nc.sync.dma_start(out=mask2_sb, in_=mask2_d.ap())
                    nc.sync.dma_start(
                        out=wo_sb, in_=wo_d.ap().rearrange("(c p) n -> p c n", p=P))

                def emit_qk(j, w):
                    ws = slice(w * W, (w + 1) * W)
                    psq = psum_b.tile([P, W], f32, tag="psb")
                    for i in range(NCI):
                        mm(psq, wh[:, 0, i, j * P:(j + 1) * P],
                           xc[i][:, ws], start=(i == 0), stop=(i == NCI - 1))
                    nc.vector.tensor_scalar_add(qT[:, j, ws], psq,
                                                bq_sb[:, j:j + 1])
                    psk = psum_b.tile([P, W], f32, tag="psb")
                    for i in range(NCI):
                        mm(psk, wh[:, 1, i, j * P:(j + 1) * P],
                           xc[i][:, ws], start=(i == 0), stop=(i == NCI - 1))
                    for a in range(2):
                        rows = slice(a * HD, (a + 1) * HD)
                        nc.vector.tensor_scalar_add(kz[rows, j, a, ws],
                                                    psk[rows, :],
                                                    bk_sb[rows, j:j + 1])

                def emit_v(it):
                    # v in row layout [t, c_out], strided into vp per head
                    psv = psum_b.tile([P, CG], f32, tag="psb")
                    for i in range(NCI):
                        mm(psv, xc[i][:, it * P:(it + 1) * P], wh[:, 2, i, :],
                           start=(i == 0), stop=(i == NCI - 1))
                    v_view = vp[:, it, :].rearrange("p (h x) -> p h x", x=VS)[:, :, 0:HD]
                    nc.vector.tensor_copy(v_view, psv.rearrange("p (h x) -> p h x", x=HD))

                # window-major so early attention windows can start while the
                # projection tail still runs
                for w in range(NW):
                    for it in range(4 * w, 4 * w + 4):
                        emit_v(it)
                    for j in range(CGP):
                        emit_qk(j, w)
            for it in range(NTT):
                ones_view = vp[:, it, :].rearrange("p (h x) -> p h x", x=VS)[:, :, HD:VS]
                nc.vector.tensor_copy(
                    ones_view, ones_sb.rearrange("p (h x) -> p h x", x=1))
            # ---- phase C+D: attention with interleaved output projection ----
            with (
                tc.tile_pool(name="attn", bufs=1) as attn,
                tc.tile_pool(name="psum_mm", bufs=2, space="PSUM") as psum_mm,
                tc.tile_pool(name="pt", bufs=8) as ptpool,
                tc.tile_pool(name="dn", bufs=4) as dnpool,
                tc.tile_pool(name="rb", bufs=4) as rbpool,
                tc.tile_pool(name="psum_av", bufs=2, space="PSUM") as psum_av,
                tc.tile_pool(name="osb", bufs=3) as opool,
                tc.tile_pool(name="psum_o", bufs=2, space="PSUM") as psum_o,
            ):
                yT = attn.tile([P, CGP, T], bf16)
                escale = 1.0 / float(np.sqrt(HD))
                for w in range(NW):
                    tq0 = w * W
                    ntk = (w + 1) * (W // P)
                    for h in range(HG):
                        j, a = h // 2, h % 2
                        ps_av = psum_av.tile([VS, W], f32, tag="av")
                        # tk blocks go in pairs sharing one psum pair-tile and
                        # one exp; a diagonal second block is extended to the
                        # pair's region and cleaned up by mask2 (128 zero cols
                        # + 128 triangular cols)
                        for x in range(ntk // 2):
                            i0, i1 = 2 * x, 2 * x + 1
                            vs0 = max(tq0, i0 * P)
                            n0 = tq0 + W - vs0
                            ps_s = psum_mm.tile([P, 2, W], f32, tag="ps")
                            pt = ptpool.tile([P, 2, W], bf16, tag="pt")
                            mm(ps_s[:, 0, W - n0:], kz[:, j, a, i0 * P:(i0 + 1) * P],
                               qT[:, j, vs0:vs0 + n0], start=True, stop=True)
                            mm(ps_s[:, 1, W - n0:], kz[:, j, a, i1 * P:(i1 + 1) * P],
                               qT[:, j, vs0:vs0 + n0], start=True, stop=True)
                            nc.scalar.activation(pt[:, :, W - n0:], ps_s[:, :, W - n0:],
                                                 AF.Exp, scale=escale)
                            if i0 * P >= tq0:  # diagonal pair
                                nc.vector.tensor_mul(
                                    pt[:, 0, W - n0:W - n0 + P],
                                    pt[:, 0, W - n0:W - n0 + P], mask_sb)
                                nc.vector.tensor_mul(
                                    pt[:, 1, W - n0:W - n0 + 2 * P],
                                    pt[:, 1, W - n0:W - n0 + 2 * P], mask2_sb)
                            mm(ps_av[:, vs0 - tq0:], vp[:, i0, h * VS:(h + 1) * VS],
                               pt[:, 0, W - n0:], start=(i0 == 0), stop=False)
                            mm(ps_av[:, vs0 - tq0:], vp[:, i1, h * VS:(h + 1) * VS],
                               pt[:, 1, W - n0:], start=False, stop=(i1 == ntk - 1))
                        dn = dnpool.tile([1, W], f32, tag="dn")
                        nc.vector.tensor_copy(dn, ps_av[HD:VS, :])
                        rb = rbpool.tile([HD, W], f32, tag="rb")
                        nc.gpsimd.partition_broadcast(rb, dn)
                        nc.vector.reciprocal_approx_fast(out=rb, in_=rb)
                        nc.vector.tensor_mul(
                            yT[a * HD:(a + 1) * HD, j, tq0:tq0 + W],
                            ps_av[0:HD, :], rb)
                    # output projection for this window
                    for m in range(C // P):
                        po = psum_o.tile([P, W], f32, tag="po")
                        for i in range(CGP):
                            mm(po, wo_sb[:, i, m * P:(m + 1) * P],
                               yT[:, i, tq0:tq0 + W],
                               start=(i == 0), stop=(i == CGP - 1))
                        ot = opool.tile([P, W], f32, tag="ot")
                        nc.vector.tensor_copy(ot, po)
                        nc.sync.dma_start(
                            out=out_d.ap()[m * P:(m + 1) * P, tq0:tq0 + W],
                            in_=ot)

    nc.compile()
    return nc


def get_nc():
    global _cached_nc
    if _cached_nc is None:
        _cached_nc = _build()
    return _cached_nc


def make_in_maps(x, Wq, bq, Wk, bk, Wv, bv, Wo, bo):
    x = np.asarray(x, np.float32)
    mask = np.triu(np.ones((P, P), np.float32))
    mask2 = np.concatenate([np.zeros((P, P), np.float32), mask], axis=1)
    in_maps = []
    for c in range(8):
        b, g = c // 2, c % 2
        cs = slice(g * CG, (g + 1) * CG)
        in_maps.append({
            "xt": np.ascontiguousarray(x[b].T.astype(ml_dtypes.bfloat16)),
            "wq": np.ascontiguousarray(
                np.asarray(Wq, np.float32)[:, cs].astype(ml_dtypes.bfloat16)),
            "wk": np.ascontiguousarray(
                np.asarray(Wk, np.float32)[:, cs].astype(ml_dtypes.bfloat16)),
            "wv": np.ascontiguousarray(
                np.asarray(Wv, np.float32)[:, cs].astype(ml_dtypes.bfloat16)),
            "wo": np.ascontiguousarray(
                np.asarray(Wo, np.float32)[cs, :].astype(ml_dtypes.bfloat16)),
            "bq": np.ascontiguousarray(
                np.asarray(bq, np.float32)[cs].reshape(CGP, P).T),
            "bk": np.ascontiguousarray(
                np.asarray(bk, np.float32)[cs].reshape(CGP, P).T),
            "mask": mask,
            "mask2": mask2,
        })
    return in_maps


def combine(results, Wv, bv, Wo, bo):
    const = (np.asarray(bv, np.float32) @ np.asarray(Wo, np.float32)
             + np.asarray(bo, np.float32))
    out = np.empty((B, T, C), np.float32)
    for b in range(B):
        acc = results[2 * b]["outp"] + results[2 * b + 1]["outp"]
        out[b] = acc.T + const[None, :]
    return out


def kernel(x, Wq, bq, Wk, bk, Wv, bv, Wo, bo):
    nc = get_nc()
    in_maps = make_in_maps(x, Wq, bq, Wk, bk, Wv, bv, Wo, bo)
    res = run_bass_kernel_spmd(nc, in_maps, core_ids=list(range(8)))
    return combine(res.results, Wv, bv, Wo, bo)
